# revision 1
# baseline (speedup 1.0000x reference)
"""MiniBatchDiscrimination Trainium2 kernel (symmetric-halved).

Reference computation:
    m = (x @ T.reshape(512, 1024)).reshape(B, 64, 16)          # [B, out, k]
    norm[i, j, o] = sum_k |m[j, o, k] - m[i, o, k]|
    o_b[i, o] = sum_j exp(-norm[i, j, o]) - 1
    out = concat([x, o_b], axis=1)                             # [B, 576]

Sharding: row-parallel with symmetry halving. Core c receives x ROTATED by
-64c rows, so its 64 rows are rows [0, 64) of its local view. Row i sums
exp(-norm) over the cyclic window j in [i+1, i+256] only (each unordered
pair lands in exactly one window, except distance-256 pairs which land in
two and are corrected separately). Every windowed term contributes to both
endpoint rows: the window-owner's sum accumulates via the ACT accum_out
(dir1), the partner row's contribution accumulates into a local ACC tensor
(dir2) that the host rotates back and sums across cores. The diagonal is
never computed, so the reference's "-1" cancels exactly.

Per-core layout:
    partitions p = (o mod 8) * 16 + k   (8 out-features x 16 kernel dims)
    MT[p, g, jj] = m_rot[jj, 8g + (p div 16), p mod 16], g = o div 8

Using |d| = 2*relu(d) - d and sum_k d_k = S_j[o] - S_i[o] (S = sum_k m):
    norm[i, j, o] = 2*sum_k relu(d) - S_j[o] + S_i[o]
  - DVE tensor_scalar(subtract, max 0) per (i, g) over the 256-wide window
    (4x bf16 DVE mode; MT_odd is a one-column-shifted copy of MT so every
    window slice starts 4B-aligned)
  - PE matmuls with a 0/1*2.0 selection matrix collapse the 16 k-partitions
    of each o into PSUM; a 9th matmul adds -S^T over the window.
  - ACT exp(-z + bias), bias = -S_i[o]; accum_out -> dir1; lagged identity
    matmuls accumulate the exp tiles into a PSUM ACC -> dir2 (lagged so the
    in-order PE queue never stalls waiting for ACT).
"""

import numpy as np

B, IN_F, OUT_F, K = 512, 512, 64, 16
NCORES = 8
RPC = B // NCORES   # rows per core = 64
NG = OUT_F // 8     # 8 column-groups of 8 out-features x 16 k = 128 partitions
W = 256             # window width
ACCW = RPC + W      # ACC columns: window cols span [1, RPC-1+W] < 320
XJ = 384            # j-columns of M actually needed per core (>= ACCW, /128)
XJT = XJ // 128     # x row-tiles to load/transpose

_cache = {}


def _build_program(repeat: int = 1, pro_repeat: int = 1):
    import concourse.bass as bass
    import concourse.bacc as bacc
    import concourse.tile as tile
    from concourse import mybir, masks

    import os as _os
    dt = mybir.dt
    f32, bf16 = dt.float32, dt.bfloat16
    Alu = mybir.AluOpType
    Act = mybir.ActivationFunctionType

    nc = bacc.Bacc(num_devices=NCORES)
    x_d = nc.dram_tensor("x", [B, IN_F], f32, kind="ExternalInput")
    t_d = nc.dram_tensor("t", [IN_F, OUT_F * K], f32, kind="ExternalInput")
    out_d = nc.dram_tensor("out", [RPC, IN_F + OUT_F], f32, kind="ExternalOutput")
    acc_d = nc.dram_tensor("acc", [OUT_F, ACCW], f32, kind="ExternalOutput")
    corr_d = nc.dram_tensor("corr", [OUT_F, RPC], f32, kind="ExternalOutput")

    from contextlib import ExitStack

    with tile.TileContext(nc) as tc, ExitStack() as ctx:
        singles = ctx.enter_context(tc.tile_pool(name="singles", bufs=1))

        ident_bf = singles.tile([128, 128], bf16, tag="ident_bf")
        masks.make_identity(nc, ident_bf[:, :])
        ident_f32 = singles.tile([128, 128], f32, tag="ident_f32")
        masks.make_identity(nc, ident_f32[:, :])

        # ZB: [128, 120] whose [:, 56-8g : 120-8g] slice is the k-collapse
        # lhsT for group g: lhsT_g[p, m] = 2.0 iff m == 8g + p//16.
        import ml_dtypes

        zb_np = np.zeros((128, 120), dtype=ml_dtypes.bfloat16)
        for p in range(128):
            zb_np[p, 56 + p // 16] = 2.0
        zb_dram = nc.inline_tensor(zb_np, name="zb_const")
        zb = singles.tile([128, 120], bf16, tag="zb")
        nc.gpsimd.dma_start(out=zb[:, :], in_=zb_dram[:, :])

        # Persistent operands
        Tsb = [singles.tile([128, OUT_F * K], bf16, tag=f"Tsb{ft}", name=f"Tsb{ft}") for ft in range(4)]
        xT = [singles.tile([128, XJ], bf16, tag=f"xT{ft}", name=f"xT{ft}") for ft in range(4)]
        MT = singles.tile([128, NG, XJ], bf16, tag="MT")
        MTodd = singles.tile([128, NG, ACCW], bf16, tag="MTodd")  # MT shifted by 1
        MTf32 = singles.tile([128, NG, RPC], f32, tag="MTf32")    # scalar operand
        SnegT = singles.tile([OUT_F, XJ], bf16, tag="SnegT")      # -S^T[o, jj]
        SmyNeg_bf = singles.tile([OUT_F, RPC], bf16, tag="SmyNeg_bf")
        SmyNeg = singles.tile([OUT_F, RPC], f32, tag="SmyNeg")    # -S_i[o]
        ACC_sb = singles.tile([OUT_F, ACCW], f32, tag="ACC_sb")   # dir2 staging
        zeros_sb = singles.tile([OUT_F, ACCW], bf16, tag="zeros_sb")
        ob_cols = singles.tile([OUT_F, RPC], f32, tag="ob_cols")  # dir1 sums
        ob_rows = singles.tile([RPC, OUT_F], f32, tag="ob_rows")

        nc.vector.memset(zeros_sb[:, :], 0.0)

        # ---------------- Prologue: load, cast, transpose, project -------
        pro = ctx.enter_context(tc.tile_pool(name="pro_sb", bufs=4))
        pps = ctx.enter_context(tc.tile_pool(name="pro_ps", bufs=2, space="PSUM"))
        pps2 = ctx.enter_context(tc.tile_pool(name="pro_ps2", bufs=1, space="PSUM"))

        for _pr in range(pro_repeat):
          for ft in range(4):
              t_stage = pro.tile([128, OUT_F * K], f32, tag="t_stage")
              for h in range(2):
                  eng = nc.sync if h == 0 else nc.gpsimd
                  eng.dma_start(
                      out=t_stage[:, 512 * h : 512 * (h + 1)],
                      in_=t_d[128 * ft : 128 * (ft + 1), 512 * h : 512 * (h + 1)],
                  )
                  nc.vector.tensor_copy(
                      out=Tsb[ft][:, 512 * h : 512 * (h + 1)],
                      in_=t_stage[:, 512 * h : 512 * (h + 1)],
                  )

          for jt in range(XJT):
              x_stage = pro.tile([128, IN_F], f32, tag="x_stage")
              for h in range(2):
                  eng = nc.sync if h == 0 else nc.gpsimd
                  eng.dma_start(
                      out=x_stage[:, 256 * h : 256 * (h + 1)],
                      in_=x_d[128 * jt : 128 * (jt + 1), 256 * h : 256 * (h + 1)],
                  )
              if jt == 0:
                  # passthrough: out[:, 0:512] = this core's rows (exact f32)
                  nc.gpsimd.dma_start(out=out_d[:, 0:IN_F], in_=x_stage[0:RPC, :])
              for ft in range(4):
                  tp = pps.tile([128, 128], f32, tag="tp")
                  nc.tensor.transpose(
                      tp[:, :], x_stage[:, 128 * ft : 128 * (ft + 1)], ident_f32[:, :]
                  )
                  nc.scalar.copy(out=xT[ft][:, 128 * jt : 128 * (jt + 1)], in_=tp[:, :])

          # MT[p, g, :] = (T_chunk_g)^T @ x^T
          for g in range(NG):
              pm = pps2.tile([128, XJ], f32, tag="pm")
              for ft in range(4):
                  nc.tensor.matmul(
                      pm[:, :],
                      lhsT=Tsb[ft][:, 128 * g : 128 * (g + 1)],
                      rhs=xT[ft][:, :],
                      start=(ft == 0),
                      stop=(ft == 3),
                  )
              nc.scalar.copy(out=MT[:, g, :], in_=pm[:, :])
              nc.vector.tensor_copy(out=MTodd[:, g, :], in_=MT[:, g, 1 : 1 + ACCW])
              nc.vector.tensor_copy(out=MTf32[:, g, :], in_=MT[:, g, 0:RPC])

          # S terms: S^T = (sum_k T)^T @ x^T — independent of the MT pipeline,
          # so SnegT is ready early. TS = T collapsed over k (DVE reduce).
          TS = [singles.tile([128, OUT_F], bf16, tag=f"TS{ft}", name=f"TS{ft}") for ft in range(4)]
          for ft in range(4):
              ts_f32 = pro.tile([128, OUT_F], f32, tag="ts_f32")
              nc.vector.tensor_reduce(
                  ts_f32[:, :],
                  Tsb[ft][:, :].rearrange("p (o k) -> p o k", k=K),
                  mybir.AxisListType.X,
                  Alu.add,
              )
              nc.vector.tensor_copy(out=TS[ft][:, :], in_=ts_f32[:, :])
          s2 = pps2.tile([OUT_F, XJ], f32, tag="pm", name="s2")
          for ft in range(4):
              nc.tensor.matmul(
                  s2[:, :],
                  lhsT=TS[ft][:, :],
                  rhs=xT[ft][:, :],
                  start=(ft == 0),
                  stop=(ft == 3),
              )
          nc.scalar.mul(SnegT[:, :], s2[:, :], -1.0)
          # bias must carry the SAME bf16 rounding as SnegT so S_j - S_i
          # cancels exactly for identical rows
          nc.scalar.mul(SmyNeg_bf[:, :], s2[:, 0:RPC], -1.0)
          nc.vector.tensor_copy(out=SmyNeg[:, :], in_=SmyNeg_bf[:, :])

        # ---------------- Main loop over this core's 64 rows -------------
        dir2_mode = "pe_lag"
        LAG = 6  # dir2 updates lag the exp by 6 iterations so PE never stalls on ACT
        GP_GROUPS = set()  # optional DVE->GPSIMD offload of relu groups (off)

        dpool = ctx.enter_context(tc.tile_pool(name="dpool", bufs=24))
        zpool = ctx.enter_context(tc.tile_pool(name="zpool", bufs=3, space="PSUM"))
        apool = ctx.enter_context(tc.tile_pool(name="apool", bufs=1, space="PSUM"))
        epool = ctx.enter_context(tc.tile_pool(name="epool", bufs=LAG + 3))

        if dir2_mode == "pe_lag":
            # dir2 accumulator in PSUM; init + accumulate all on PE
            ACC = apool.tile([OUT_F, ACCW], f32, tag="ACC")
            nc.tensor.matmul(
                ACC[:, :],
                lhsT=ident_bf[0:OUT_F, 0:OUT_F],
                rhs=zeros_sb[:, :],
                start=True,
                stop=(repeat == 0),
                skip_group_check=True,
            )
        else:
            ACC = ACC_sb
            nc.vector.memset(ACC[:, :], 0.0)

        def emit_dir2(li, le, last):
            llo = li % RPC + 1
            if dir2_mode == "pe_lag":
                nc.tensor.matmul(
                    ACC[:, llo : llo + W],
                    lhsT=ident_bf[0:OUT_F, 0:OUT_F],
                    rhs=le[:, :],
                    start=False,
                    stop=last,
                    skip_group_check=True,
                )
            elif dir2_mode == "dve_lag":
                nc.vector.tensor_add(
                    ACC[:, llo : llo + W], ACC[:, llo : llo + W], le[:, :]
                )

        e_hist = []
        iters = list(range(RPC)) * repeat
        for it_idx, i in enumerate(iters):
            lo = i + 1  # window = [lo, lo + W)
            z = zpool.tile([OUT_F, W], f32, tag="z")
            # z = -S^T over the window first: its input is ready from the
            # prologue, so PE can open each z group without waiting on DVE
            nc.tensor.matmul(
                z[:, :],
                lhsT=ident_bf[0:OUT_F, 0:OUT_F],
                rhs=SnegT[:, lo : lo + W],
                start=True,
                stop=False,
            )
            for g in range(NG):
                r_g = dpool.tile([128, W], bf16, tag="d")
                if lo % 2 == 0:
                    win = MT[:, g, lo : lo + W]
                else:
                    win = MTodd[:, g, lo - 1 : lo - 1 + W]
                eng = nc.gpsimd if (GP_GROUPS and g in GP_GROUPS) else nc.vector
                eng.tensor_scalar(
                    r_g[:, :],
                    win,
                    MTf32[:, g, i : i + 1],
                    0.0,
                    Alu.subtract,
                    Alu.max,
                )
                nc.tensor.matmul(
                    z[:, :],
                    lhsT=zb[:, 56 - 8 * g : 120 - 8 * g],
                    rhs=r_g[:, :],
                    start=False,
                    stop=(g == NG - 1),
                )
            e = epool.tile([OUT_F, W], bf16, tag="e")
            nc.scalar.activation(
                out=e[:, :],
                in_=z[:, :],
                func=Act.Exp,
                scale=-1.0,
                bias=SmyNeg[:, i : i + 1],
                accum_out=ob_cols[:, i : i + 1],
            )
            if dir2_mode != "none":
                e_hist.append((i, e))
                if len(e_hist) > LAG:
                    li, le = e_hist.pop(0)
                    emit_dir2(li, le, False)
        # flush remaining dir2 updates
        for n, (li, le) in enumerate(e_hist):
            if dir2_mode != "none":
                emit_dir2(li, le, n == len(e_hist) - 1)
        e_hist = []

        # ------------- distance-256 correction pairs (qq, qq+256) --------
        d0 = dpool.tile([128, NG, RPC], bf16, tag="d", name="d0")
        nc.vector.tensor_sub(d0[:, :, :], MT[:, :, 0:RPC], MT[:, :, W : W + RPC])
        r1 = dpool.tile([128, NG, RPC], bf16, tag="d", name="r1")
        nc.vector.tensor_relu(r1[:, :, :], d0[:, :, :])
        r2 = dpool.tile([128, NG, RPC], bf16, tag="d", name="r2")
        nc.vector.tensor_scalar(
            r2[:, :, :], d0[:, :, :], -1.0, 0.0, Alu.mult, Alu.max
        )
        ad = dpool.tile([128, NG, RPC], bf16, tag="d", name="ad")
        nc.vector.tensor_add(ad[:, :, :], r1[:, :, :], r2[:, :, :])
        z3 = zpool.tile([OUT_F, RPC], f32, tag="z3", bufs=1)
        for g in range(NG):
            nc.tensor.matmul(
                z3[:, :],
                lhsT=zb[:, 56 - 8 * g : 120 - 8 * g],
                rhs=ad[:, g, :],
                start=(g == 0),
                stop=(g == NG - 1),
            )
        corr_sb = singles.tile([OUT_F, RPC], f32, tag="corr_sb")
        nc.scalar.activation(
            out=corr_sb[:, :], in_=z3[:, :], func=Act.Exp, scale=-0.5
        )
        nc.gpsimd.dma_start(out=corr_d[:, :], in_=corr_sb[:, :])

        # ---------------- Epilogue: stores ------------------------------
        for bi in range(2):
            for bj in range(2):
                nc.vector.transpose(
                    ob_rows[32 * bi : 32 * bi + 32, 32 * bj : 32 * bj + 32],
                    ob_cols[32 * bj : 32 * bj + 32, 32 * bi : 32 * bi + 32],
                )
        nc.gpsimd.dma_start(out=out_d[:, IN_F : IN_F + OUT_F], in_=ob_rows[:, :])
        if dir2_mode == "pe_lag":
            nc.scalar.copy(out=ACC_sb[:, :], in_=ACC[:, :])
        nc.gpsimd.dma_start(out=acc_d[:, :], in_=ACC_sb[:, :])

    nc.compile()
    if not nc.is_finalized():
        nc.finalize()
    return nc


def _get_program():
    if "nc" not in _cache:
        _cache["nc"] = _build_program()
    return _cache["nc"]


def kernel(x: np.ndarray, T: np.ndarray) -> np.ndarray:
    import os

    from concourse.bass_utils import run_bass_kernel_spmd

    nc = _get_program()
    x = np.ascontiguousarray(x, dtype=np.float32)
    t2 = np.ascontiguousarray(T, dtype=np.float32).reshape(IN_F, OUT_F * K)
    in_maps = [
        {"x": np.ascontiguousarray(np.roll(x, -RPC * c, axis=0)), "t": t2}
        for c in range(NCORES)
    ]
    try:
        res = run_bass_kernel_spmd(nc, in_maps, core_ids=list(range(NCORES)))
    except ModuleNotFoundError:
        # BASS_TRACE requested but the axon NTFF hook (antenv) is absent in
        # this container — retry with tracing disabled.
        os.environ["BASS_NEVER_TRACE"] = "1"
        res = run_bass_kernel_spmd(nc, in_maps, core_ids=list(range(NCORES)))
    _cache["last_results"] = res

    out_full = np.empty((B, IN_F + OUT_F), np.float32)
    ob = np.zeros((B, OUT_F), np.float64)
    for c in range(NCORES):
        r = res.results[c]
        out_full[RPC * c : RPC * (c + 1), :IN_F] = r["out"][:, :IN_F]
        ob[RPC * c : RPC * (c + 1)] += r["out"][:, IN_F:]          # dir1
        tmp = np.zeros((OUT_F, B), np.float64)
        tmp[:, :ACCW] = r["acc"]
        ob += np.roll(tmp, RPC * c, axis=1).T                      # dir2
    for c in range(4):  # distance-256 corrections, canonical q in [0, 256)
        corr = res.results[c]["corr"].T                            # [RPC, OUT_F]
        ob[RPC * c : RPC * (c + 1)] -= corr
        ob[RPC * c + W : RPC * (c + 1) + W] -= corr
    out_full[:, IN_F:] = ob.astype(np.float32)
    return out_full


if __name__ == "__main__":
    rng = np.random.default_rng(0)
    x = rng.standard_normal((B, IN_F), dtype=np.float32)
    T = rng.standard_normal((IN_F, OUT_F, K), dtype=np.float32)
    out = kernel(x, T)
    print("out shape:", out.shape, out.dtype)
    print("x passthrough exact:", np.array_equal(out[:, :IN_F], x))
    print("o_b stats:", np.abs(out[:, IN_F:]).max())



# revision 2
# speedup vs baseline: 1.5431x; 1.5431x over previous
"""MiniBatchDiscrimination Trainium2 kernel (symmetric-halved, fp8-DR PE).

Reference computation:
    m = (x @ T.reshape(512, 1024)).reshape(B, 64, 16)          # [B, out, k]
    norm[i, j, o] = sum_k |m[j, o, k] - m[i, o, k]|
    o_b[i, o] = sum_j exp(-norm[i, j, o]) - 1
    out = concat([x, o_b], axis=1)                             # [B, 576]

Row-parallel with symmetry halving: core c gets x rotated by -64c rows; its
64 rows are local rows [0, 64). Row i sums exp(-norm) over the cyclic window
j in [i+1, i+256]; each windowed term feeds both endpoint rows (dir1 via ACT
accum_out, dir2 via a PSUM ACC), distance-256 pairs are double-counted
across cores and corrected on the host. |d| = 2*relu(d) - d with
S = sum_k m: norm = 2*sum_k relu(d) - S_j + S_i.

Implementation notes:
  - m is kept in fp16; DVE/Pool/ACT emit fp16 relu tiles (DVE tensor_scalar
    runs in 4x mode). PE reads ONLY THE HIGH BYTES of those fp16 tiles as
    fp8e5m2 (fp16 truncated to 2 mantissa bits -- an exact dtype prefix) and
    collapses the 16 k-partitions of TWO 8-out-feature groups at a time with
    one DoubleRow fp8 matmul (0.5 cycles/col, 2 k-tiles): 4 matmuls/row.
  - dir2: exp outputs land in paired slots [64, 2, 258] with one column of
    zero padding around each row's window, offset by row parity; a single
    DoubleRow identity matmul accumulates BOTH rows into ACC.
  - the host passes x both as-is and pre-transposed (layout marshaling,
    like the np.roll); T loads in group-pair column chunks so projected M
    groups land progressively.
  - relu emission is staggered: groups 4-5 of row r are emitted 6
    iterations late and groups 6-7 (plus the whole z/exp/ACC pipeline) 12
    iterations late, so no engine queue ever head-blocks on T data that is
    still in flight.
  - the distance-256 correction values are the exact e-values already
    computed in each row's window (col 256), extracted with strided copies.
"""

import numpy as np

B, IN_F, OUT_F, K = 512, 512, 64, 16
NCORES = 8
RPC = B // NCORES   # rows per core = 64
NG = OUT_F // 8     # 8 groups of 8 out-features x 16 k = 128 partitions
W = 256             # window width
ACCW = RPC + W      # ACC columns
MTW = ACCW          # MT columns used (320)
XJ = 384            # j-columns of x^T needed (>= MTW)
NPAIR = RPC // 2    # 32 row pairs
EW = W + 2          # e pair-tile columns (window + 2 pads)
S2 = 8              # relu stagger for pair 2 (groups 4, 5)
S3 = 8              # relu stagger for pair 3 + z/exp/ACC machinery
LAGP = 2            # ACC update lag in pairs
ZBUFS = 6           # z psum ring depth
ACT_ROWS = (0, 3, 6)  # i%8 values whose g1 relu runs on ACT

_cache = {}


def _g1_engine(i):
    """Engine for the group-1 relu of row i: 2=ACT, 1=Pool."""
    return 2 if i % 8 in ACT_ROWS else 1


def _build_program(repeat: int = 1, pro_repeat: int = 1):
    import concourse.bass as bass
    import concourse.bacc as bacc
    import concourse.tile as tile
    import ml_dtypes
    from concourse import mybir, masks

    dt = mybir.dt
    f32, bf16, fp16, e5 = dt.float32, dt.bfloat16, dt.float16, dt.float8e5
    Alu = mybir.AluOpType
    Act = mybir.ActivationFunctionType
    DR = mybir.MatmulPerfMode.DoubleRow

    nc = bacc.Bacc(num_devices=NCORES)
    x_d = nc.dram_tensor("x", [RPC, IN_F], f32, kind="ExternalInput")
    xt_d = nc.dram_tensor("xt", [IN_F, B], fp16, kind="ExternalInput")
    t_d = nc.dram_tensor("t", [IN_F, OUT_F * K], bf16, kind="ExternalInput")
    out_d = nc.dram_tensor("out", [RPC, IN_F + OUT_F], f32, kind="ExternalOutput")
    acc_d = nc.dram_tensor("acc", [OUT_F, ACCW], f32, kind="ExternalOutput")
    corr_d = nc.dram_tensor("corr", [OUT_F, RPC], f32, kind="ExternalOutput")

    # ---- constant weights -------------------------------------------------
    # zbDR[p, g2, t, :]: 2.0 at col 16*g2 + 8*t + p//16 -- k-collapse weights
    # for the group pair (2*g2, 2*g2 + t) as the two DoubleRow k-tiles.
    import ml_dtypes as mld

    zb_np = np.zeros((128, 4, 2, OUT_F), dtype=mld.float8_e5m2)
    for p in range(128):
        for g2 in range(4):
            for t in range(2):
                zb_np[p, g2, t, 16 * g2 + 8 * t + p // 16] = 2.0
    # identDR[p, t, p] = 1.0: sums both e k-tiles into ACC.
    iddr_np = np.zeros((OUT_F, 2, OUT_F), dtype=mld.float8_e5m2)
    for p in range(OUT_F):
        iddr_np[p, 0, p] = 1.0
        iddr_np[p, 1, p] = 1.0
    # selS[p, g, 8g + p//16] = -1.0: -S^T accumulation from M.
    selS_np = np.zeros((128, NG, OUT_F), dtype=mld.bfloat16)
    for p in range(128):
        for g in range(NG):
            selS_np[p, g, 8 * g + p // 16] = -1.0

    from contextlib import ExitStack

    with tile.TileContext(nc) as tc, ExitStack() as ctx:
        singles = ctx.enter_context(tc.tile_pool(name="singles", bufs=1))
        pro_ctx = ExitStack()

        # PE warm-up at t~0; more warm-keeper matmuls are chained onto DMA
        # arrivals below so the p-state ramp survives the load window.
        warm = singles.tile([64, 64], bf16, tag="warm")
        nc.vector.memset(warm[:, :], 0.0)
        wps = pro_ctx.enter_context(tc.tile_pool(name="wps", bufs=1, space="PSUM"))
        wz = wps.tile([64, 8], f32, tag="wz")
        nc.tensor.matmul(wz[:, :], lhsT=warm[:, :], rhs=warm[:, 0:8],
                         start=True, stop=True)
        warm_e = singles.tile([64, 8], fp16, tag="warm_e")
        nc.scalar.activation(out=warm_e[:, :], in_=warm[:, 0:8], func=Act.Exp,
                             scale=-1.0)

        def pe_keepwarm(dep_ap):
            nc.tensor.matmul(wz[:, :], lhsT=dep_ap, rhs=dep_ap[:, 0:8],
                             start=True, stop=True, skip_group_check=True)

        zb_dram = nc.inline_tensor(zb_np, name="zb_const")
        zbDR = singles.tile([128, 4, 2, OUT_F], e5, tag="zbDR")
        nc.gpsimd.dma_start(out=zbDR[:, :, :, :], in_=zb_dram[:, :, :, :])
        iddr_dram = nc.inline_tensor(iddr_np, name="iddr_const")
        idDR = singles.tile([OUT_F, 2, OUT_F], e5, tag="idDR")
        nc.gpsimd.dma_start(out=idDR[:, :, :], in_=iddr_dram[:, :, :])
        selS_dram = nc.inline_tensor(selS_np, name="selS_const")
        selS = singles.tile([128, NG, OUT_F], bf16, tag="selS")
        nc.gpsimd.dma_start(out=selS[:, :, :], in_=selS_dram[:, :, :])

        ident_bf = singles.tile([128, 128], bf16, tag="ident_bf")
        masks.make_identity(nc, ident_bf[:, :])

        # Persistent operands
        Tcast = [singles.tile([128, 4, 256], bf16, tag=f"Tc{c}", name=f"Tc{c}")
                 for c in range(4)]
        xT = singles.tile([128, 4, XJ], fp16, tag="xT")
        MT = [singles.tile([128, MTW], fp16, tag=f"MT{g}", name=f"MT{g}")
              for g in range(NG)]
        MTf32 = [singles.tile([128, RPC], f32, tag=f"MTf32_{g}", name=f"MTf32_{g}")
                 for g in range(NG)]
        MTneg1 = singles.tile([128, RPC], f32, tag="MTneg1")
        SnegT = singles.tile([OUT_F, MTW], bf16, tag="SnegT")
        SmyNeg = singles.tile([OUT_F, RPC], f32, tag="SmyNeg")
        ACC_sb = singles.tile([OUT_F, ACCW], f32, tag="ACC_sb")
        zeros_sb = singles.tile([OUT_F, ACCW], bf16, tag="zeros_sb")
        ob_cols = singles.tile([OUT_F, RPC], f32, tag="ob_cols")
        ob_rows = singles.tile([RPC, OUT_F], f32, tag="ob_rows")
        corr_sb = singles.tile([OUT_F, RPC], f32, tag="corr_sb")
        xpass = singles.tile([RPC, IN_F], f32, tag="xpass")
        # e pair tiles: 2 big tiles x 16 pairs x 2 slots x EW cols
        ebig = [singles.tile([OUT_F, NPAIR // 2, 2, EW], fp16, tag=f"ebig{h}",
                             name=f"ebig{h}") for h in range(2)]
        # relu tile rings, one per group pair (sized to cover the stagger)
        NR = [S3 + 4, S3 + 8, S3 - S2 + 4, 4]
        rings = [[singles.tile([128, 2, W], fp16, tag=f"r{p}_{n}", name=f"r{p}_{n}")
                  for n in range(NR[p])] for p in range(4)]

        nc.vector.memset(zeros_sb[:, :], 0.0)
        for h in range(2):
            nc.gpsimd.memset(ebig[h][:, :, 0, 0], 0.0)
            nc.gpsimd.memset(ebig[h][:, :, 0, W + 1], 0.0)
            nc.gpsimd.memset(ebig[h][:, :, 1, 0:2], 0.0)

        # ---------------- Loads + projection (pipelined) -------------------
        pro = ctx.enter_context(tc.tile_pool(name="pro_sb", bufs=5))
        pps2 = pro_ctx.enter_context(tc.tile_pool(name="pro_ps2", bufs=2, space="PSUM"))

        assert pro_repeat == 1
        # x^T (host-marshaled fp16) in one DMA; T (host-marshaled bf16) in
        # group-pair column chunks so projected M groups land progressively.
        xt_src = xt_d[:, 0:XJ].rearrange("(ft p) q -> p ft q", ft=4)
        nc.sync.dma_start(out=xT[:, :, :], in_=xt_src)
        for c in range(4):
            src = t_d[:, 256 * c : 256 * (c + 1)].rearrange("(ft p) q -> p ft q", ft=4)
            eng = nc.sync if c % 2 == 0 else nc.scalar
            eng.dma_start(out=Tcast[c][:, :, :], in_=src)
        pe_keepwarm(xT[0:64, 0, 0:64])

        def cp_act(out, in_):
            nc.scalar.copy(out=out, in_=in_)

        def cp_dve(out, in_):
            nc.vector.tensor_copy(out=out, in_=in_)

        s2 = pps2.tile([OUT_F, MTW], f32, tag="s2")

        def emit_pair_chain(c):
            """Project groups (2c, 2c+1), copy M out, accumulate -S^T."""
            cp = cp_dve if c < 2 else cp_act
            pe_keepwarm(Tcast[c][0:64, 0, 0:64])
            for gg in range(2):
                g = 2 * c + gg
                pm = pps2.tile([128, MTW], f32, tag="pm")
                for ft in range(4):
                    nc.tensor.matmul(
                        pm[:, :],
                        lhsT=Tcast[c][:, ft, 128 * gg : 128 * (gg + 1)],
                        rhs=xT[:, ft, 0:MTW],
                        start=(ft == 0),
                        stop=(ft == 3),
                    )
                cp(MT[g][:, :], pm[:, :])
                cp(MTf32[g][:, :], pm[:, 0:RPC])
                nc.tensor.matmul(
                    s2[:, :],
                    lhsT=selS[:, g, :],
                    rhs=MT[g][:, :],
                    start=(g == 0),
                    stop=(g == NG - 1),
                )

        emit_pair_chain(0)
        emit_pair_chain(1)
        # negated scalar for the ACT-relu bias (group 1)
        nc.vector.tensor_scalar_mul(MTneg1[:, :], MTf32[1][:, :], -1.0)

        # ---------------- Main loop (staggered pipeline) -------------------
        n_rows = RPC * repeat
        pend = []
        pro_done = False
        zpool = apool = ACC = None

        for it in range(n_rows + S3):
            # pair-2/3 prologue chains interleave where their T data lands
            if it == 2:
                emit_pair_chain(2)
            if it == 5:
                emit_pair_chain(3)
                nc.scalar.copy(out=SnegT[:, :], in_=s2[:, :])
                nc.scalar.copy(out=SmyNeg[:, :], in_=SnegT[:, 0:RPC])
                pro_ctx.close()
                pro_done = True
                zpool = ctx.enter_context(
                    tc.tile_pool(name="zpool", bufs=ZBUFS, space="PSUM"))
                apool = ctx.enter_context(
                    tc.tile_pool(name="apool", bufs=1, space="PSUM"))
                ACC = apool.tile([OUT_F, ACCW], f32, tag="ACC")
                nc.tensor.matmul(
                    ACC[:, :],
                    lhsT=ident_bf[0:OUT_F, 0:OUT_F],
                    rhs=zeros_sb[:, :],
                    start=True,
                    stop=(repeat == 0),
                    skip_group_check=True,
                )

            # ---- relu emission, staggered per pair ----
            if it < n_rows:
                i = it % RPC
                lo = i + 1
                r2 = rings[0][it % NR[0]]
                nc.gpsimd.tensor_scalar(
                    r2[:, 0, :], MT[0][:, lo : lo + W], MTf32[0][:, i : i + 1],
                    0.0, Alu.subtract, Alu.max)
                if _g1_engine(i) == 2:
                    nc.scalar.activation(
                        out=r2[:, 1, :], in_=MT[1][:, lo : lo + W],
                        func=Act.Relu, scale=1.0, bias=MTneg1[:, i : i + 1])
                else:
                    nc.gpsimd.tensor_scalar(
                        r2[:, 1, :], MT[1][:, lo : lo + W], MTf32[1][:, i : i + 1],
                        0.0, Alu.subtract, Alu.max)
                r2 = rings[1][it % NR[1]]
                for t in range(2):
                    g = 2 + t
                    nc.vector.tensor_scalar(
                        r2[:, t, :], MT[g][:, lo : lo + W], MTf32[g][:, i : i + 1],
                        0.0, Alu.subtract, Alu.max)
            if S2 <= it < n_rows + S2:
                i2 = (it - S2) % RPC
                lo2 = i2 + 1
                r2 = rings[2][(it - S2) % NR[2]]
                for t in range(2):
                    g = 4 + t
                    nc.vector.tensor_scalar(
                        r2[:, t, :], MT[g][:, lo2 : lo2 + W], MTf32[g][:, i2 : i2 + 1],
                        0.0, Alu.subtract, Alu.max)
            if it < S3:
                continue

            # ---- row r = it - S3: pair-3 relus + z + exp + ACC ----
            r = it - S3
            i = r % RPC
            lo = i + 1
            r2 = rings[3][r % NR[3]]
            for t in range(2):
                g = 6 + t
                nc.vector.tensor_scalar(
                    r2[:, t, :], MT[g][:, lo : lo + W], MTf32[g][:, i : i + 1],
                    0.0, Alu.subtract, Alu.max)

            z = zpool.tile([OUT_F, W], f32, tag="z")
            # ring index: pair p of row r was emitted at iteration r (pairs
            # 0, 1), r + S2 - S2 = r (pair 2, indexed by emission row), r
            # (pair 3): all rings are indexed by the row number r.
            for g2 in range(4):
                rr = rings[g2][r % NR[g2]]
                if g2 == 3:
                    nc.tensor.matmul(
                        z[:, :],
                        lhsT=ident_bf[0:OUT_F, 0:OUT_F],
                        rhs=SnegT[:, lo : lo + W],
                        start=False,
                        stop=False,
                    )
                nc.tensor.matmul(
                    z[:, :],
                    lhsT=zbDR[:, g2, :, :],
                    rhs=rr[:, :, :].bitcast(e5)[:, :, 1::2],
                    start=(g2 == 0),
                    stop=(g2 == 3),
                    perf_mode=DR,
                )
            q = i // 2
            t = i % 2
            eslice = ebig[q // 16][:, q % 16, t, 1 + t : 1 + t + W]
            nc.scalar.activation(
                out=eslice,
                in_=z[:, :],
                func=Act.Exp,
                scale=-1.0,
                bias=SmyNeg[:, i : i + 1],
                accum_out=ob_cols[:, i : i + 1],
            )
            if r == 32 and repeat == 1:
                # rows 0-31 exps done: extract their distance-256 e values
                # corr_sb[o, i] = ebig[0][o, (i//2)%16, i%2, 256+i%2]
                for tt in range(2):
                    nc.vector.tensor_copy(
                        out=corr_sb[:, tt : 32 : 2],
                        in_=ebig[0][:, :, tt, 256 + tt])
            if t == 1:
                pend.append(q)
                if len(pend) > LAGP:
                    q0 = pend.pop(0)
                    nc.tensor.matmul(
                        ACC[:, 2 * q0 : 2 * q0 + EW],
                        lhsT=idDR[:, :, :],
                        rhs=ebig[q0 // 16][:, q0 % 16, :, :].bitcast(e5)[:, :, 1::2],
                        start=False,
                        stop=False,
                        skip_group_check=True,
                        perf_mode=DR,
                    )
        for n, q0 in enumerate(pend):
            nc.tensor.matmul(
                ACC[:, 2 * q0 : 2 * q0 + EW],
                lhsT=idDR[:, :, :],
                rhs=ebig[q0 // 16][:, q0 % 16, :, :].bitcast(e5)[:, :, 1::2],
                start=False,
                stop=(n == len(pend) - 1),
                skip_group_check=True,
                perf_mode=DR,
            )
        pend = []

        # ---- distance-256 corrections (half 1; half 0 emitted mid-loop) ---
        for t in range(2):
            src = ebig[1][:, :, t, 256 + t]
            dst = corr_sb[:, 32 + t : 64 : 2]
            nc.vector.tensor_copy(out=dst, in_=src)
        nc.sync.dma_start(out=corr_d[:, :], in_=corr_sb[:, :])

        # ---------------- Epilogue: stores ---------------------------------
        nc.scalar.dma_start(out=xpass[:, :], in_=x_d[:, :])
        nc.scalar.dma_start(out=out_d[:, 0:IN_F], in_=xpass[:, :])
        for bi in range(2):
            for bj in range(2):
                nc.vector.transpose(
                    ob_rows[32 * bi : 32 * bi + 32, 32 * bj : 32 * bj + 32],
                    ob_cols[32 * bj : 32 * bj + 32, 32 * bi : 32 * bi + 32],
                )
        nc.sync.dma_start(out=out_d[:, IN_F : IN_F + OUT_F], in_=ob_rows[:, :])
        nc.scalar.copy(out=ACC_sb[:, :], in_=ACC[:, :])
        nc.scalar.dma_start(out=acc_d[:, :], in_=ACC_sb[:, :])
        assert pro_done

    nc.compile()
    if not nc.is_finalized():
        nc.finalize()
    return nc


def _get_program():
    if "nc" not in _cache:
        _cache["nc"] = _build_program()
    return _cache["nc"]


def kernel(x: np.ndarray, T: np.ndarray) -> np.ndarray:
    import os

    from concourse.bass_utils import run_bass_kernel_spmd

    nc = _get_program()
    x = np.ascontiguousarray(x, dtype=np.float32)
    t2 = np.ascontiguousarray(T, dtype=np.float32).reshape(IN_F, OUT_F * K)
    in_maps = []
    import ml_dtypes as mld

    t2_bf = t2.astype(mld.bfloat16)
    for c in range(NCORES):
        xr = np.roll(x, -RPC * c, axis=0)
        in_maps.append({
            "x": np.ascontiguousarray(xr[:RPC]),
            "xt": np.ascontiguousarray(xr.T.astype(np.float16)),
            "t": t2_bf,
        })
    try:
        res = run_bass_kernel_spmd(nc, in_maps, core_ids=list(range(NCORES)))
    except ModuleNotFoundError:
        os.environ["BASS_NEVER_TRACE"] = "1"
        res = run_bass_kernel_spmd(nc, in_maps, core_ids=list(range(NCORES)))
    _cache["last_results"] = res

    out_full = np.empty((B, IN_F + OUT_F), np.float32)
    ob = np.zeros((B, OUT_F), np.float64)
    for c in range(NCORES):
        r = res.results[c]
        out_full[RPC * c : RPC * (c + 1), :IN_F] = r["out"][:, :IN_F]
        ob[RPC * c : RPC * (c + 1)] += r["out"][:, IN_F:]          # dir1
        tmp = np.zeros((OUT_F, B), np.float64)
        tmp[:, :ACCW] = r["acc"]
        ob += np.roll(tmp, RPC * c, axis=1).T                      # dir2
    for c in range(4):  # distance-256 corrections, canonical q in [0, 256)
        corr = res.results[c]["corr"].T                            # [RPC, OUT_F]
        ob[RPC * c : RPC * (c + 1)] -= corr
        ob[RPC * c + W : RPC * (c + 1) + W] -= corr
    out_full[:, IN_F:] = ob.astype(np.float32)
    return out_full


if __name__ == "__main__":
    rng = np.random.default_rng(0)
    x = rng.standard_normal((B, IN_F), dtype=np.float32)
    T = rng.standard_normal((IN_F, OUT_F, K), dtype=np.float32)
    out = kernel(x, T)
    print("out shape:", out.shape, out.dtype)
    print("x passthrough exact:", np.array_equal(out[:, :IN_F], x))
    print("o_b stats:", np.abs(out[:, IN_F:]).max())
